# revision 79
# baseline (speedup 1.0000x reference)
"""Trainium2 Bass kernel for dense causal(-penalty) attention.

Problem: x[4,4096,512], Wq/Wk/Wv[512,64] -> out[4,4096,64]
  q,k,v = x@W;  scores = (1/8) q k^T;  masked = scores - 20*strict_upper;
  out = softmax(masked) @ v

The -20 causal penalty makes masked weights ~2e-9 relative, so blocks that
are entirely above the diagonal are skipped outright (they contribute
nothing at the 2e-2 tolerance). To keep a single SPMD program while
balancing the causal triangle, the two cores of a batch split the KEY dim
by 256-block parity: core h of batch b owns key pairs {256*(2t+h)} and all
8 query windows, computing a partial softmax numerator+denominator
[65, 4096] (65th row = sum of weights). The host adds the two partials and
divides.

Uniformity trick: the host ships x^T with 256-column blocks interleaved
[own_0, other_0, own_1, other_1, ...], so "own keys" are the even 256-slots
and query window w is always permuted columns [512w, 512w+512) — all
per-core differences live in the data (the permutation and a shipped
[128,1024] triangle pattern with exp(-20) in masked cells).

Per core: 36 units (window w gets pairs t=0..w; t=w is the triangular
boundary). Each unit: packed ST = K_chunk^T @ Q_window on both PE
row-groups, one exp ACT call on [128,1024], boundary units multiply by the
pattern on DVE, then PV accumulates [65,512] into PSUM; finished windows
DMA straight from PSUM to DRAM.
"""
import math
import ml_dtypes
import numpy as np

import concourse.mybir as mybir
import concourse.tile as tile
from concourse import bacc
from concourse.bass_utils import run_bass_kernel_spmd
from concourse.masks import make_identity

F32 = mybir.dt.float32
F32R = mybir.dt.float32r
BF16 = mybir.dt.bfloat16
AF = mybir.ActivationFunctionType

B, S, D, H = 4, 4096, 512, 64
QW = 512              # q window width
NW = S // QW          # 8 windows per core
NCHUNK = D // 128     # 4 contraction chunks
LKC = 16              # local k chunks (own 2048 keys)
NEG = -20.0
EPS = math.exp(NEG)
SCALE = 0.125
NU = NW * (NW + 1) // 2   # 36 units

_CACHE = {}

# unit m -> (window w, local pair t<=w). The last window processes its
# boundary pair FIRST so the kernel tail ends on a full (mult-free) unit.
_UNITS = [(w, t) for w in range(NW)
          for t in ([w] + list(range(w)) if w == NW - 1 else range(w + 1))]
_WFIRST = {w: w * (w + 1) // 2 for w in range(NW)}
_WLAST = {w: w * (w + 1) // 2 + w for w in range(NW)}


def _build(repeat=1, diag_half_exp=False, diag_no_dma=False):
    nc = bacc.Bacc("TRN2", target_bir_lowering=False, debug=False, num_devices=8)
    xt_d = nc.dram_tensor("xt", [D, S], BF16, kind="ExternalInput").ap()
    # host pre-arranges each weight to [128, (c h)] so its DMA is contiguous
    # 512B rows (128B runs of the naive layout run below SDMA line rate)
    w_d = nc.dram_tensor("wqkv", [3 * 128, NCHUNK * H], BF16,
                         kind="ExternalInput").ap()
    pat_d = nc.dram_tensor("pat", [128, 2 * QW], BF16, kind="ExternalInput").ap()
    out_d = nc.dram_tensor("outp", [H + 1, S], F32, kind="ExternalOutput").ap()

    with tile.TileContext(nc) as tc:
        with tc.tile_pool(name="big", bufs=1) as big, \
             tc.tile_pool(name="cst", bufs=1) as cst, \
             tc.tile_pool(name="pt", bufs=3) as ptp, \
             tc.tile_pool(name="stp", bufs=2, space="PSUM") as stp, \
             tc.tile_pool(name="otp", bufs=2, space="PSUM") as otp:
          for _rep in range(repeat):
            # ---- DMA emission order = HWDGE drain order; one merged DMA per
            # slice (issue cost is ~565ns SP-sequencer time per dma_start) --
            wt = cst.tile([128, 3 * NCHUNK * H], BF16, tag="wqkv", name="wqkv")
            WOF = {"wq": 0, "wk": 1, "wv": 2}

            def wsl(nm, dc):
                o = NCHUNK * H * WOF[nm] + H * dc
                return wt[:, o:o + H]

            xta = big.tile([128, NCHUNK * S], BF16, tag="xta", name="xta")

            def xts(dc, lo, ln):
                return xta[:, S * dc + lo:S * dc + lo + ln]

            patw = cst.tile([128, 2 * QW], BF16, tag="patw")
            patb = cst.tile([128, 2 * QW], F32R, tag="pat")

            def load_x(lo, ln):
                nc.sync.dma_start(
                    xta[:].rearrange("p (c s) -> p c s", s=S)[:, :, lo:lo + ln],
                    xt_d.rearrange("(c p) s -> p c s", p=128)[:, :, lo:lo + ln])

            def load_w(nm, eng):
                o = WOF[nm]
                eng.dma_start(
                    wt[:, NCHUNK * H * o:NCHUNK * H * (o + 1)],
                    w_d[128 * o:128 * (o + 1), :])

            load_w("wk", nc.sync)
            nc.gpsimd.dma_start(
                xta[:].rearrange("p (c s) -> p c s", s=S)[:, :, 0:256],
                xt_d.rearrange("(c p) s -> p c s", p=128)[:, :, 0:256])
            load_w("wq", nc.sync)
            load_x(256, 256)
            load_w("wv", nc.sync)
            load_x(512, 256)
            load_x(768, 256)
            nc.sync.dma_start(patw[:], pat_d)
            tail_slices = [(1024, 1024), (2048, 1024), (3072, 1024)]
            if diag_no_dma:
                tail_slices = []  # timing diagnostic only — wrong numerics
            for lo, ln in tail_slices:
                load_x(lo, ln)

            # ---- constants ----
            ident = cst.tile([128, 128], F32, tag="id")
            ones16 = cst.tile([128, LKC], F32, tag="ones")
            qtd = big.tile([64, S], F32R, tag="qtd")
            ktd = big.tile([64, LKC * 128], F32R, tag="ktd")
            vts = big.tile([64, LKC * 128], F32, tag="vts")
            vsb = big.tile([128, LKC * (H + 1)], F32R, tag="v")
            otsb = big.tile([H + 1, S], F32, tag="otsb")

            def emit_constants():
                make_identity(nc, ident[:])
                nc.gpsimd.memset(ones16[:], 1.0)
                nc.gpsimd.tensor_copy(vsb[:, H::H + 1], ones16[:])
                nc.gpsimd.tensor_copy(patb[:], patw[:])

            # ---- projections --------------------------------------------
            # wq window w: Q^T[:, 512w:512w+512] on partitions 0..63, then
            # SBUF->SBUF DMA duplication onto 64..127 for packed ST.
            pend = {}

            def wq_half(w, first):
                if first:
                    ps = stp.tile([64, QW], F32, tag="vps", name="psq")
                    pend[("q", w)] = ps
                    dcs = range(0, 2)
                else:
                    ps = pend.pop(("q", w))
                    dcs = range(2, 4)
                for dc in dcs:
                    nc.tensor.matmul(
                        ps[:], wsl("wq", dc),
                        xts(dc, QW * w, QW),
                        start=(dc == 0), stop=(dc == NCHUNK - 1))
                if first:
                    return
                sl = slice(QW * w, QW * w + QW)
                nc.vector.tensor_copy(qtd[:, sl], ps[:])

            # wk/VT group g covers own slots {2g, 2g+1} = perm columns
            # [1024g, 1024g+256) and [1024g+512, 1024g+768).
            def kv_half(nm, g, first):
                if first:
                    ps = stp.tile([64, QW], F32, tag="vps", name=f"ps{nm}")
                    pend[(nm, g)] = ps
                    sls = [0]
                else:
                    ps = pend.pop((nm, g))
                    sls = [1]
                for sl in sls:
                    for dc in range(NCHUNK):
                        nc.tensor.matmul(
                            ps[:, 256 * sl:256 * sl + 256],
                            wsl(nm, dc),
                            xts(dc, 1024 * g + 512 * sl, 256),
                            start=(dc == 0), stop=(dc == NCHUNK - 1))
                    # copy this half immediately: key pair 2g+sl becomes
                    # ready without waiting for the other half's xt columns
                    dst = slice(QW * g + 256 * sl, QW * g + 256 * sl + 256)
                    src = ps[:, 256 * sl:256 * sl + 256]
                    if nm == "wk":
                        nc.vector.tensor_copy(ktd[:, dst], src)
                    else:
                        nc.vector.tensor_copy(vts[:, dst], src)

            # transpose V^T group g (4 local chunks) into k-major vsb
            def vtrans(g):
                vtp = stp.tile([128, 4 * H], F32, tag="vps", name="vtp")
                for i in range(4):
                    nc.tensor.transpose(
                        vtp[:, H * i:H * i + H],
                        vts[:, QW * g + 128 * i:QW * g + 128 * i + 128],
                        ident[0:H, 0:H])
                nc.vector.tensor_copy(
                    vsb[:].rearrange("p (c e) -> p c e", e=H + 1)
                    [:, 4 * g:4 * g + 4, 0:H],
                    vtp[:].rearrange("p (c e) -> p c e", e=H))

            # ---- attention units ----------------------------------------
            ot_of = {}

            def emit_st(m):
                w, t = _UNITS[m]
                if m == _WFIRST[w]:
                    ot_of[w] = otp.tile([H + 1, QW], F32, tag="otps",
                                        name=f"otps{w}")
                stt = stp.tile([128, 2 * QW], F32, tag="st", name="stt")
                for j in range(2):
                    kc = 2 * t + j
                    nc.tensor.matmul(
                        stt[:, QW * j:QW * j + QW],
                        ktd[:, 128 * kc:128 * kc + 128],
                        qtd[:, QW * w:QW * w + QW],
                        start=True, stop=True)
                return stt

            def emit_exp(m, stt):
                w, t = _UNITS[m]
                pt = ptp.tile([128, 2 * QW], F32R, tag="pt", name="ptt")
                if diag_half_exp:
                    # timing diagnostic only — wrong numerics (pt half stale)
                    nc.scalar.activation(pt[:, 0:QW], stt[:, 0:QW], AF.Exp,
                                         bias=0.0, scale=SCALE)
                    if m < 3:
                        nc.scalar.activation(pt[:, QW:], stt[:, QW:], AF.Exp,
                                             bias=0.0, scale=SCALE)
                elif m == 0 or m == NU - 1:
                    # split first/last exps (and any boundary mult) per half:
                    # halves start as soon as their ST half is ready (head)
                    # and release PV sooner (tail)
                    for j in range(2):
                        hs = slice(QW * j, QW * j + QW)
                        nc.scalar.activation(pt[:, hs], stt[:, hs], AF.Exp,
                                             bias=0.0, scale=SCALE)
                        if t == w:
                            nc.vector.tensor_mul(pt[:, hs], pt[:, hs],
                                                 patb[:, hs])
                    return pt
                else:
                    nc.scalar.activation(pt[:], stt[:], AF.Exp,
                                         bias=0.0, scale=SCALE)
                if t == w:
                    nc.vector.tensor_mul(pt[:], pt[:], patb[:])
                return pt

            def emit_pv(m, pt):
                w, t = _UNITS[m]
                otps = ot_of[w]
                for j in range(2):
                    kc = 2 * t + j
                    # boundary second chunk: window cols [0,128) are dead on
                    # both cores (pattern is EPS there) — skip them, except
                    # when this unit opens the window (cols must initialize)
                    lo = 128 if (t == w and j == 1 and m != _WFIRST[w]) else 0
                    nc.tensor.matmul(
                        otps[:, lo:QW],
                        vsb[:, (H + 1) * kc:(H + 1) * (kc + 1)],
                        pt[:, QW * j + lo:QW * j + QW],
                        start=(m == _WFIRST[w] and j == 0),
                        stop=(m == _WLAST[w] and j == 1))
                if m == _WLAST[w]:
                    sl = slice(QW * w, QW * w + QW)
                    nc.vector.tensor_copy(otsb[:, sl], otps[:])
                    if w >= NW - 2:
                        # last two windows ship immediately on the idle SP
                        # ring (tail latency)
                        nc.sync.dma_start(out_d[:, sl], otsb[:, sl])
                    elif w % 2 == 1:
                        # earlier windows ship two per DMA
                        sl2 = slice(QW * (w - 1), QW * w + QW)
                        nc.sync.dma_start(out_d[:, sl2], otsb[:, sl2])

            # ---- prologue: only what ST(0) and ST(1) need ----
            kv_half("wk", 0, True)
            kv_half("wk", 0, False)
            wq_half(0, True)
            wq_half(0, False)
            emit_constants()
            st0 = emit_st(0)
            wq_half(1, True)
            wq_half(1, False)


            # interleave plan: step -> list of (fn, args); emission deadlines:
            # wq w by plan[w(w+1)/2 - 2], wk g by plan[2g^2+3g-2],
            # wv/vtrans g by plan[2g^2+3g+1] (vtrans after its wv halves)
            plan = {
                0: [(kv_half, ("wv", 0, True)), (kv_half, ("wv", 0, False))],
                1: [(vtrans, (0,)), (wq_half, (2, True)), (wq_half, (2, False))],
                2: [(kv_half, ("wk", 1, True))],
                3: [(kv_half, ("wk", 1, False))],
                4: [(wq_half, (3, True)), (wq_half, (3, False))],
                5: [(kv_half, ("wv", 1, True))],
                6: [(kv_half, ("wv", 1, False)), (vtrans, (1,))],
                7: [(wq_half, (4, True))],
                8: [(wq_half, (4, False))],
                9: [(kv_half, ("wk", 2, True))],
                10: [(kv_half, ("wk", 2, False))],
                11: [(wq_half, (5, True))],
                12: [(wq_half, (5, False))],
                13: [(kv_half, ("wv", 2, True))],
                14: [(kv_half, ("wv", 2, False)), (vtrans, (2,))],
                17: [(wq_half, (6, True))],
                18: [(wq_half, (6, False))],
                22: [(kv_half, ("wk", 3, True))],
                23: [(kv_half, ("wk", 3, False))],
                24: [(kv_half, ("wv", 3, True)), (wq_half, (7, True))],
                25: [(kv_half, ("wv", 3, False)), (wq_half, (7, False))],
                26: [(vtrans, (3,))],
            }

            # ---- software-pipelined main loop ----
            st_cur = st0
            pt_prev = None
            for m in range(NU):
                st_next = emit_st(m + 1) if m + 1 < NU else None
                pt_cur = emit_exp(m, st_cur)
                for fn, args in plan.get(m, []):
                    fn(*args)
                if pt_prev is not None:
                    emit_pv(m - 1, pt_prev)
                st_cur, pt_prev = st_next, pt_cur
            emit_pv(NU - 1, pt_prev)
    nc.compile()
    return nc


def _perm_oq(h):
    """orig q offset within a window for permuted col j."""
    j = np.arange(QW)
    if h == 0:
        return j
    return np.where(j < 256, 256 + j, j - 256)


def make_in_maps(x, Wq, Wk, Wv):
    in_maps = []
    for c in range(8):
        b, h = c // 2, c % 2
        xt = x[b].T  # [D, S]
        cols = np.empty(S, np.int64)
        for u in range(NW):
            cols[512 * u:512 * u + 256] = np.arange(
                512 * u + 256 * h, 512 * u + 256 * h + 256)
            cols[512 * u + 256:512 * u + 512] = np.arange(
                512 * u + 256 * (1 - h), 512 * u + 256 * (1 - h) + 256)
        xtp = np.ascontiguousarray(xt[:, cols]).astype(ml_dtypes.bfloat16)
        kk = np.arange(128)
        oq = _perm_oq(h)
        pat = np.empty((128, 2 * QW), np.float32)
        for jh in range(2):
            ok = 256 * h + 128 * jh + kk
            pat[:, QW * jh:QW * jh + QW] = np.where(
                oq[None, :] >= ok[:, None], 1.0, EPS).astype(np.float32)
        wqkv = np.concatenate(
            [np.asarray(w, np.float32).reshape(NCHUNK, 128, H)
             .transpose(1, 0, 2).reshape(128, NCHUNK * H)
             for w in (Wq, Wk, Wv)], axis=0).astype(ml_dtypes.bfloat16)
        in_maps.append({"xt": xtp, "pat": pat.astype(ml_dtypes.bfloat16),
                        "wqkv": wqkv})
    return in_maps


def kernel(x, Wq, Wk, Wv):
    x = np.ascontiguousarray(np.asarray(x, dtype=np.float32))
    Wq = np.asarray(Wq, dtype=np.float32)
    Wk = np.asarray(Wk, dtype=np.float32)
    Wv = np.asarray(Wv, dtype=np.float32)

    if "nc" not in _CACHE:
        _CACHE["nc"] = _build()
    nc = _CACHE["nc"]

    in_maps = make_in_maps(x, Wq, Wk, Wv)
    res = run_bass_kernel_spmd(nc, in_maps, list(range(8)))

    # host combine: unpermute each core's partial, add the pair, divide
    inv = {}
    for h in range(2):
        oq = _perm_oq(h)
        iv = np.empty(S, np.int64)
        for w in range(NW):
            iv[QW * w + oq] = QW * w + np.arange(QW)
        inv[h] = iv
    out = np.empty((B, S, H), dtype=np.float32)
    for b in range(B):
        full = res.results[2 * b]["outp"][:, inv[0]] \
            + res.results[2 * b + 1]["outp"][:, inv[1]]
        out[b] = (full[0:H] / full[H:H + 1]).T
    return out
